# revision 69
# baseline (speedup 1.0000x reference)
"""AreaAttention Trainium2 kernel: 8-core data-parallel over batch.

Each core processes one [512, 64, 64] image through:
  qk = SiLU(BN(conv1x1(x)));  v = SiLU(BN(conv1x1(x)))
  pp = SiLU(BN(conv3x3(v)))
  area attention (4 windows of 1024 tokens, 8 heads of dim 64) over qk/v
  y = SiLU(BN(conv1x1(attn_out + pp)))

BN scales folded into conv weights on host; weights pre-transposed to
[K, M] lhsT layouts; convs run bf16 with fp32 PSUM, 1024-wide matmul
outputs (2 PSUM banks) to amortize weight-load bubbles, 2048-wide
SiLU tiles to amortize ActivationEngine fixed overhead.

Attention per (window, head): S^T = K^T Q as one 1024-wide bf16 matmul
per 128-key chunk; exp on ScalarE over [128, 2048] jc-pair tiles
(scale=1/8 folded in) writes fp8 probs; attn@V runs fp8 DoubleRow with
256-deep contraction (two key chunks per matmul: lhsT [128, 2, 65],
rhs [128, 2, 1024]) with a ones column in Vaug so row 64 of the output
is the softmax denominator; reciprocal is computed in a [128, 8]
reshape (DVE reciprocal cost scales with columns) and broadcast across
partitions via a DRAM bounce.

conv3x3 runs on flat [C, 4096] maps with shifted contiguous slices per
tap; two extra copies of v with boundary columns zeroed handle the
horizontal pad, range-clipping handles the vertical pad. conv3x3 + the
final 1x1 conv interleave with the attention windows so the
TensorEngine always has dense work.
"""

import numpy as np

import concourse.bacc as bacc
import concourse.bass as bass
from concourse import mybir
from concourse.tile import TileContext
from concourse.masks import make_identity

P = 128
C = 512
CI = C // P          # 4 input-channel chunks
OCQK = 2 * C // P    # 8 qk output chunks
OC = C // P          # 4 output chunks
HW = 4096            # 64*64 tokens
WIN = 4              # area windows
NW = HW // WIN       # 1024 tokens per window
HEADS = 8
HD = 64
JC = NW // P         # 8 key chunks per window
EPS = 1e-5
FP32 = mybir.dt.float32
BF16 = mybir.dt.bfloat16
FP8 = mybir.dt.float8e4
SILU = mybir.ActivationFunctionType.Silu
EXP = mybir.ActivationFunctionType.Exp
TANH = mybir.ActivationFunctionType.Tanh
ADD = mybir.AluOpType.add
MULT = mybir.AluOpType.mult
DR = mybir.MatmulPerfMode.DoubleRow

# taps ordered center-first so a full-range matmul opens each PSUM group
TAPS = [(1, 1)] + [(ky, kx) for ky in range(3) for kx in range(3) if (ky, kx) != (1, 1)]


def _phase_a(nc, env):
    """x in; v conv -> v_mid (bf16); qk conv -> qk_dram (bf16)."""
    psAa = env['psAa']; xpool = env['xpool']; stpool = env['stpool']
    x_ext = env['x_ext']; v_bf = env['v_bf']; qk_bf = env['qk_bf']
    v_mid = env['v_mid']; qk_dram = env['qk_dram']
    bv_sb = env['bv_sb']; bqk_sb = env['bqk_sb']

    x_ci = []
    for ci in range(CI):
        t = xpool.tile([P, HW], BF16, tag=f"x{ci}")
        nc.sync.dma_start(out=t[:], in_=x_ext[:, ci, :])
        x_ci.append(t)

    def conv_tile(w_sb, oc, half2):
        # one [128, 4, 512] PSUM tile covering tokens [2048*half2, +2048)
        ps = psAa.tile([P, 4, 512], FP32, tag="s")
        for half in range(4):
            n0 = half2 * 2048 + half * 512
            for ci in range(CI):
                nc.tensor.matmul(
                    ps[:, half, :],
                    w_sb[:, ci, oc * P : (oc + 1) * P],
                    x_ci[ci][:, n0 : n0 + 512],
                    start=(ci == 0),
                    stop=(ci == CI - 1),
                    skip_group_check=True,
                )
        return ps

    # first half of the token range first, so windows 0-1 of the merged
    # phase can start while the second half is still convolving
    for half2 in range(2):
        for oc in range(OC):
            ps = conv_tile(v_bf, oc, half2)
            nc.scalar.activation(
                v_mid[:, oc, half2 * 2048 : (half2 + 1) * 2048],
                ps[:].rearrange("p a b -> p (a b)"),
                SILU,
                bias=bv_sb[:, oc : oc + 1],
            )
        for oc in range(OCQK):
            ps = conv_tile(qk_bf, oc, half2)
            st = stpool.tile([P, 2048], BF16, tag="st")
            nc.scalar.activation(
                st[:], ps[:].rearrange("p a b -> p (a b)"),
                SILU, bias=bqk_sb[:, oc : oc + 1],
            )
            nc.sync.dma_start(
                out=qk_dram[oc * P : (oc + 1) * P,
                            half2 * 2048 : (half2 + 1) * 2048],
                in_=st[:],
            )


def _phase_merged(nc, env):
    pepool = env['pepool']; vaugp = env['vaugp']; a8p = env['a8p']
    qkp = env['qkp']; recipp = env['recipp']; srowp = env['srowp']; tmpp = env['tmpp']
    ppw = env['ppw']; ystp = env['ystp']; attnw = env['attnw']
    pestp = env['pestp']; prstp = env['prstp']; gatep = env['gatep']
    psA = env['psA']; psO = env['psO']; psum_mm = env['psum_mm']
    vlr = env['vlr']; v_mid = env['v_mid']; ident = env['ident']; qk_dram = env['qk_dram']
    dram2 = env['dram2']; pew_ext = env['pew_ext']; pr_bf = env['pr_bf']
    bpe_sb = env['bpe_sb']; bpr_sb = env['bpr_sb']; out_ext = env['out_ext']

    # pr conv results of window g-1, silu'd while window g's exps run
    pending_pr = []

    def flush_pr(gate):
        for pr_st, oc, nch in pending_pr:
            yst = ystp.tile([P, 512], FP32, tag="yst")
            bias = gate[:, oc : oc + 1] if gate is not None else bpr_sb[:, oc : oc + 1]
            nc.scalar.activation(yst[:], pr_st[:], SILU, bias=bias)
            nc.sync.dma_start(
                out=out_ext[oc * P : (oc + 1) * P, nch * 512 : (nch + 1) * 512],
                in_=yst[:],
            )
        pending_pr.clear()

    for g in range(WIN):
        w = g
        # ---- V-aug (fp8) [128 keys, h, jc, 80] with ones column ----
        # row stride 80 so the DoubleRow lhsT kt-dim stride (80 B) is
        # 16-byte aligned (ISA s3_lw_dual_fp8 restriction)
        attn_w = attnw.tile([P, OC, NW], BF16, tag="attnw")
        vaug = vaugp.tile([P, HEADS, JC, 80], FP8, tag="vaug")
        nc.vector.memset(vaug[:, :, :, HD : HD + 1], 1.0)
        for jc in range(JC):
            t0 = w * NW + jc * P
            pt = psum_mm.tile([P, CI, P], BF16, tag="mm")
            for ci in range(CI):
                nc.tensor.matmul(
                    pt[:, ci, :], v_mid[:, ci, t0 : t0 + P], ident[:],
                    is_transpose=True, skip_group_check=True,
                )
            nc.vector.tensor_copy(
                vaug[:, :, jc, 0:HD],
                pt[:].rearrange("p c (s d) -> p (c s) d", s=2),
            )

        # ---- conv3x3 matmuls (oc 0-1 before the heads: they are
        # long-ready PE work that bridges the window boundary while the
        # first exps drain behind the gated silu batch) ----
        og = NW * g - 128                      # tile origin in token space
        ca, cb = max(0, og), min(HW, NW * (g + 1) + 128)
        v_l = vlr.tile([P, CI, NW + 256], BF16, tag="vl")
        v_r = vlr.tile([P, CI, NW + 256], BF16, tag="vr")
        nc.vector.tensor_copy(v_l[:, :, ca - og : cb - og], v_mid[:, :, ca:cb])
        nc.vector.tensor_copy(v_r[:, :, ca - og : cb - og], v_mid[:, :, ca:cb])
        rl = v_l[:].rearrange("p c (r w) -> p c r w", w=64)
        rr = v_r[:].rearrange("p c (r w) -> p c r w", w=64)
        nc.vector.memset(rl[:, :, (ca - og) // 64 : (cb - og) // 64, 63:64], 0)
        nc.vector.memset(rr[:, :, (ca - og) // 64 : (cb - og) // 64, 0:1], 0)
        vtap = {0: v_l, 1: None, 2: v_r}
        pp_w = ppw.tile([P, OC, NW], BF16, tag="ppw")

        def conv3x3_oc(oc):
            pe_sl = pepool.tile([P, CI * 9 * P], BF16, tag="pe")
            nc.sync.dma_start(out=pe_sl[:], in_=pew_ext[oc])
            pe_v = pe_sl[:].rearrange("p (c t o) -> p c t o", c=CI, t=9)
            pe_st = pestp.tile([P, 2, 512], BF16, tag="pest")
            for nch in (2 * g, 2 * g + 1):
                n0 = nch * 512
                ps = psum_mm.tile([P, 512], FP32, tag="mm")
                mms = []
                for ky, kx in TAPS:
                    s = (ky - 1) * 64 + (kx - 1)
                    lo = max(0, -s - n0)
                    hi = min(512, HW - s - n0)
                    for ci in range(CI):
                        if kx == 1:
                            rsl = v_mid[:, ci, n0 + s + lo : n0 + s + hi]
                        else:
                            rsl = vtap[kx][
                                :, ci, n0 + s + lo - og : n0 + s + hi - og
                            ]
                        mms.append((
                            ps[:, lo:hi],
                            pe_v[:, ci, ky * 3 + kx, :],
                            rsl,
                        ))
                for i, (o, l, r) in enumerate(mms):
                    nc.tensor.matmul(
                        o, l, r,
                        start=(i == 0),
                        stop=(i == len(mms) - 1),
                        skip_group_check=True,
                    )
                # evacuate PSUM right away (DVE) so conv matmuls never
                # block on the gated silu
                nc.vector.tensor_copy(pe_st[:, nch - 2 * g, :], ps[:])
            return pe_st

        for h in range(HEADS):
            q_t = qkp.tile([HD, NW], BF16, tag="q")
            nc.sync.dma_start(
                out=q_t[:],
                in_=qk_dram[h * HD : (h + 1) * HD, w * NW : (w + 1) * NW],
            )
            k_t = qkp.tile([HD, NW], BF16, tag="k")
            nc.sync.dma_start(
                out=k_t[:],
                in_=qk_dram[C + h * HD : C + (h + 1) * HD, w * NW : (w + 1) * NW],
            )

            ps_o = psO.tile([HD + 1, NW], FP32, tag="o")
            a8 = None
            for jp in range(JC // 2):
                ps_s = psA.tile([P, 2, NW], FP32, tag="s")
                for jcl in range(2):
                    for half in range(2):
                        nc.tensor.matmul(
                            ps_s[:, jcl, half * 512 : (half + 1) * 512],
                            k_t[:, (2 * jp + jcl) * P : (2 * jp + jcl + 1) * P],
                            q_t[:, half * 512 : (half + 1) * 512],
                            start=True,
                            stop=True,
                            skip_group_check=True,
                        )
                a8 = a8p.tile([P, 2, NW], FP8, tag="a8")
                nc.scalar.activation(
                    a8[:].rearrange("p a b -> p (a b)"),
                    ps_s[:].rearrange("p a b -> p (a b)"),
                    EXP, scale=0.125,
                )
                for half in range(2):
                    nc.tensor.matmul(
                        ps_o[:, half * 512 : (half + 1) * 512],
                        vaug[:, h, 2 * jp : 2 * jp + 2, 0 : HD + 1],
                        a8[:, :, half * 512 : (half + 1) * 512],
                        start=(jp == 0),
                        stop=(jp == JC // 2 - 1),
                        perf_mode=DR,
                        skip_group_check=True,
                    )

            # evict attnV PSUM to SBUF right away so the next head's
            # matmuls aren't blocked behind the normalize chain
            srow = srowp.tile([HD + 1, NW], FP32, tag="srow")
            nc.vector.tensor_copy(srow[:], ps_o[:])
            rrow = dram2.tile([NW], FP32, tag="rrow")
            nc.sync.dma_start(out=rrow[:], in_=srow[HD : HD + 1, :])
            r128 = recipp.tile([P, JC], FP32, tag="r128")
            nc.sync.dma_start(
                out=r128[:], in_=rrow[:].rearrange("(p c) -> p c", p=P)
            )
            nc.vector.reciprocal(r128[:], r128[:])
            rrec = dram2.tile([NW], FP32, tag="rrec")
            nc.sync.dma_start(
                out=rrec[:].rearrange("(p c) -> p c", p=P), in_=r128[:]
            )
            rbc = recipp.tile([HD, NW], FP32, tag="rbc")
            nc.sync.dma_start(
                out=rbc[:], in_=rrec[:].unsqueeze(0).partition_broadcast(HD)
            )
            if h % 2 == 0:
                nc.vector.tensor_mul(
                    attn_w[0:HD, h // 2, :], srow[0:HD, :], rbc[:]
                )
            else:
                tmp = tmpp.tile([HD, NW], BF16, tag="tmp")
                nc.vector.tensor_mul(tmp[:], srow[0:HD, :], rbc[:])
                nc.sync.dma_start(
                    out=attn_w[HD : 2 * HD, h // 2, :], in_=tmp[:]
                )
            if g == WIN - 1 and h == 5:
                # last window: gate the conv silus on head 5's exps so the
                # final pp_w -> z -> pr chain overlaps heads 6-7 instead of
                # trailing the kernel (costs ~2 extra act-table loads)
                gate_pe_l = gatep.tile([P, OC], FP32, tag="gpe")
                nc.vector.scalar_tensor_tensor(
                    gate_pe_l[:], a8[:, 1, 0:OC], 0.0, bpe_sb[:], MULT, ADD
                )
                gate_pr_l = gatep.tile([P, OC], FP32, tag="gpr")
                nc.vector.scalar_tensor_tensor(
                    gate_pr_l[:], a8[:, 1, 0:OC], 0.0, bpr_sb[:], MULT, ADD
                )
            if h == HEADS - 1:
                a8_last = a8

        # gate tiles: read one element of the window's final exp output so
        # the conv silus below schedule after all of this window's exps
        # (keeps Exp/Silu act-table loads to ~2 per window). The last
        # window runs ungated so its conv tail overlaps the attention.
        if g == WIN - 1:
            gate_pr, gate_pe = gate_pr_l, gate_pe_l
        else:
            gate_pr = gatep.tile([P, OC], FP32, tag="gpr")
            nc.vector.scalar_tensor_tensor(
                gate_pr[:], a8_last[:, 1, 0:OC], 0.0, bpr_sb[:], MULT, ADD
            )
            gate_pe = gatep.tile([P, OC], FP32, tag="gpe")
            nc.vector.scalar_tensor_tensor(
                gate_pe[:], a8_last[:, 1, 0:OC], 0.0, bpe_sb[:], MULT, ADD
            )
        flush_pr(gate_pr)

        # ---- conv3x3 matmuls, then all gated silus ----
        pe_sts = [conv3x3_oc(oc) for oc in range(OC)]
        for oc, pe_st in enumerate(pe_sts):
            nc.scalar.activation(
                pp_w[:, oc, :],
                pe_st[:].rearrange("p a b -> p (a b)"),
                SILU,
                bias=gate_pe[:, oc : oc + 1],
            )

        # ---- pr conv for this window's tokens ----
        for nloc in range(2):
            nch = 2 * g + nloc
            z = ppw.tile([P, CI, 512], BF16, tag="z")
            nc.vector.tensor_add(
                z[:],
                attn_w[:, :, nloc * 512 : (nloc + 1) * 512],
                pp_w[:, :, nloc * 512 : (nloc + 1) * 512],
            )
            for oc in range(OC):
                ps = psum_mm.tile([P, 512], FP32, tag="mm")
                for ci in range(CI):
                    nc.tensor.matmul(
                        ps[:],
                        pr_bf[:, ci, oc * P : (oc + 1) * P],
                        z[:, ci, :],
                        start=(ci == 0),
                        stop=(ci == CI - 1),
                    )
                pr_st = prstp.tile([P, 512], BF16, tag="prst")
                nc.vector.tensor_copy(pr_st[:], ps[:])
                pending_pr.append((pr_st, oc, nch))

    # last window's pr silus run ungated at the end
    flush_pr(None)


def _build():
    nc = bacc.Bacc(None, target_bir_lowering=False, debug=False)

    x_ext = nc.declare_dram_parameter("x", [P, CI, HW], BF16, isOutput=False)
    qkw_ext = nc.declare_dram_parameter("qk_wt", [P, CI, 2 * C], BF16, isOutput=False)
    vw_ext = nc.declare_dram_parameter("v_wt", [P, CI, C], BF16, isOutput=False)
    pew_ext = nc.declare_dram_parameter("pe_wt", [OC, P, CI * 9 * P], BF16, isOutput=False)
    prw_ext = nc.declare_dram_parameter("pr_wt", [P, CI, C], BF16, isOutput=False)
    bqk_ext = nc.declare_dram_parameter("b_qk", [P, OCQK], FP32, isOutput=False)
    bv_ext = nc.declare_dram_parameter("b_v", [P, OC], FP32, isOutput=False)
    bpe_ext = nc.declare_dram_parameter("b_pe", [P, OC], FP32, isOutput=False)
    bpr_ext = nc.declare_dram_parameter("b_pr", [P, OC], FP32, isOutput=False)
    out_ext = nc.declare_dram_parameter("out", [C, HW], FP32, isOutput=True)

    with TileContext(nc) as tc:
        with (
            tc.tile_pool(name="const", bufs=1) as const_pool,
            tc.tile_pool(name="persist", bufs=1) as persist,
            tc.tile_pool(name="dram", bufs=1, space="DRAM") as dram,
            tc.tile_pool(name="dram2", bufs=3, space="DRAM") as dram2,
        ):
            ident = const_pool.tile([P, P], BF16)
            make_identity(nc, ident)

            bqk_sb = const_pool.tile([P, OCQK], FP32)
            nc.sync.dma_start(out=bqk_sb[:], in_=bqk_ext[:])
            bv_sb = const_pool.tile([P, OC], FP32)
            nc.sync.dma_start(out=bv_sb[:], in_=bv_ext[:])
            bpe_sb = const_pool.tile([P, OC], FP32)
            nc.sync.dma_start(out=bpe_sb[:], in_=bpe_ext[:])
            bpr_sb = const_pool.tile([P, OC], FP32)
            nc.sync.dma_start(out=bpr_sb[:], in_=bpr_ext[:])

            qk_bf = persist.tile([P, CI, 2 * C], BF16)
            nc.sync.dma_start(out=qk_bf[:], in_=qkw_ext[:])
            v_bf = persist.tile([P, CI, C], BF16)
            nc.sync.dma_start(out=v_bf[:], in_=vw_ext[:])
            pr_bf = persist.tile([P, CI, C], BF16)
            nc.sync.dma_start(out=pr_bf[:], in_=prw_ext[:])

            # v feature map (flat) -- attention V source and conv3x3 center
            v_mid = persist.tile([P, OC, HW], BF16)

            qk_dram = dram.tile([2 * C, HW], BF16)

            # phase A gets all 8 PSUM banks for double-buffered
            # [128, 2, 1024] conv tiles
            with (
                tc.tile_pool(name="psAa", bufs=2, space="PSUM") as psAa,
                tc.tile_pool(name="xpool", bufs=1) as xpool,
                tc.tile_pool(name="stpool", bufs=2) as stpool,
            ):
                _phase_a(nc, locals())

            from contextlib import ExitStack

            with ExitStack() as es:
                pools = dict(
                    psum_mm=("PSUM", 2), psA=("PSUM", 1), psO=("PSUM", 1),
                    vlr=(None, 1), attnw=(None, 1), pepool=(None, 2),
                    vaugp=(None, 2), a8p=(None, 4), qkp=(None, 3),
                    srowp=(None, 2), recipp=(None, 2), tmpp=(None, 2),
                    ppw=(None, 2), ystp=(None, 4), pestp=(None, 8),
                    prstp=(None, 10), gatep=(None, 2),
                )
                pvars = {}
                for pname, (space, bufs) in pools.items():
                    kw = {"space": space} if space else {}
                    pvars[pname] = es.enter_context(
                        tc.tile_pool(name=pname, bufs=bufs, **kw)
                    )
                env = dict(locals())
                env.update(pvars)
                _phase_merged(nc, env)

    nc.compile()
    return nc


_NC_CACHE = {}


def _get_nc():
    if "nc" not in _NC_CACHE:
        _NC_CACHE["nc"] = _build()
    return _NC_CACHE["nc"]


def _make_in_maps(inputs):
    import ml_dtypes

    bf16 = ml_dtypes.bfloat16
    x = np.asarray(inputs["x"], dtype=np.float32)          # [8, 512, 64, 64]
    B = x.shape[0]

    def fold(wname, gname, bname, mname, vname):
        g = np.asarray(inputs[gname], np.float32)
        b = np.asarray(inputs[bname], np.float32)
        m = np.asarray(inputs[mname], np.float32)
        v = np.asarray(inputs[vname], np.float32)
        s = g / np.sqrt(v + EPS)
        w = np.asarray(inputs[wname], np.float32)
        return s, (b - m * s).astype(np.float32), w

    s_qk, b_qk, qk_w = fold("qk_w", "qk_g", "qk_b", "qk_rm", "qk_rv")
    s_v, b_v, v_w = fold("v_w", "v_g", "v_b", "v_rm", "v_rv")
    s_pe, b_pe, pe_w = fold("pe_w", "pe_g", "pe_b", "pe_rm", "pe_rv")
    s_pr, b_pr, pr_w = fold("pr_w", "pr_g", "pr_b", "pr_rm", "pr_rv")

    def lhst(w_scaled, o_dim):
        # [O, C] scaled -> [128, CI, O] bf16 (partition = c % 128)
        wt = w_scaled.T.reshape(CI, P, o_dim).transpose(1, 0, 2)
        return np.ascontiguousarray(wt.astype(bf16))

    qk_wt = lhst(qk_w * s_qk[:, None], 2 * C)
    v_wt = lhst(v_w * s_v[:, None], C)
    pr_wt = lhst(pr_w * s_pr[:, None], C)

    # pe: [O, C, 3, 3] -> per oc chunk: [128(c%128), CI, 9, 128(o)] bf16
    pe = (pe_w * s_pe[:, None, None, None]).transpose(2, 3, 1, 0)  # ky,kx,c,o
    pe = pe.reshape(9, CI, P, OC, P)            # tap, ci, p, oc, op
    pe = pe.transpose(3, 2, 1, 0, 4)            # oc, p, ci, tap, op
    pe_wt = np.ascontiguousarray(pe.reshape(OC, P, CI * 9 * P).astype(bf16))

    def bias_r(b, n):
        return np.ascontiguousarray(b.reshape(n, P).T)

    shared = {
        "qk_wt": qk_wt, "v_wt": v_wt, "pe_wt": pe_wt, "pr_wt": pr_wt,
        "b_qk": bias_r(b_qk, OCQK), "b_v": bias_r(b_v, OC),
        "b_pe": bias_r(b_pe, OC), "b_pr": bias_r(b_pr, OC),
    }
    xs = x.reshape(B, CI, P, HW).transpose(0, 2, 1, 3).astype(bf16)
    return [
        {"x": np.ascontiguousarray(xs[i]), **shared}
        for i in range(B)
    ]


def kernel(**inputs):
    from concourse.bass_utils import run_bass_kernel_spmd

    in_maps = _make_in_maps(inputs)
    B = len(in_maps)
    nc = _get_nc()
    res = run_bass_kernel_spmd(nc, in_maps, core_ids=list(range(B)))
    out = np.stack([res.results[i]["out"] for i in range(B)], axis=0)
    return out.reshape(B, C, 64, 64).astype(np.float32)


# revision 70
# speedup vs baseline: 1.0285x; 1.0285x over previous
"""AreaAttention Trainium2 kernel: 8-core data-parallel over batch.

Each core processes one [512, 64, 64] image through:
  qk = SiLU(BN(conv1x1(x)));  v = SiLU(BN(conv1x1(x)))
  pp = SiLU(BN(conv3x3(v)))
  area attention (4 windows of 1024 tokens, 8 heads of dim 64) over qk/v
  y = SiLU(BN(conv1x1(attn_out + pp)))

BN scales folded into conv weights on host; weights pre-transposed to
[K, M] lhsT layouts; convs run bf16 with fp32 PSUM, 1024-wide matmul
outputs (2 PSUM banks) to amortize weight-load bubbles, 2048-wide
SiLU tiles to amortize ActivationEngine fixed overhead.

Attention per (window, head): S^T = K^T Q as one 1024-wide bf16 matmul
per 128-key chunk; exp on ScalarE over [128, 2048] jc-pair tiles
(scale=1/8 folded in) writes fp8 probs; attn@V runs fp8 DoubleRow with
256-deep contraction (two key chunks per matmul: lhsT [128, 2, 65],
rhs [128, 2, 1024]) with a ones column in Vaug so row 64 of the output
is the softmax denominator; reciprocal is computed in a [128, 8]
reshape (DVE reciprocal cost scales with columns) and broadcast across
partitions via a DRAM bounce.

conv3x3 runs on flat [C, 4096] maps with shifted contiguous slices per
tap; two extra copies of v with boundary columns zeroed handle the
horizontal pad, range-clipping handles the vertical pad. conv3x3 + the
final 1x1 conv interleave with the attention windows so the
TensorEngine always has dense work.
"""

import numpy as np

import concourse.bacc as bacc
import concourse.bass as bass
from concourse import mybir
from concourse.tile import TileContext
from concourse.masks import make_identity

P = 128
C = 512
CI = C // P          # 4 input-channel chunks
OCQK = 2 * C // P    # 8 qk output chunks
OC = C // P          # 4 output chunks
HW = 4096            # 64*64 tokens
WIN = 4              # area windows
NW = HW // WIN       # 1024 tokens per window
HEADS = 8
HD = 64
JC = NW // P         # 8 key chunks per window
EPS = 1e-5
FP32 = mybir.dt.float32
BF16 = mybir.dt.bfloat16
FP8 = mybir.dt.float8e4
SILU = mybir.ActivationFunctionType.Silu
EXP = mybir.ActivationFunctionType.Exp
TANH = mybir.ActivationFunctionType.Tanh
ADD = mybir.AluOpType.add
MULT = mybir.AluOpType.mult
DR = mybir.MatmulPerfMode.DoubleRow

# taps ordered center-first so a full-range matmul opens each PSUM group
TAPS = [(1, 1)] + [(ky, kx) for ky in range(3) for kx in range(3) if (ky, kx) != (1, 1)]


def _phase_a(nc, env):
    """x in; v conv -> v_mid (bf16); qk conv -> qk_dram (bf16)."""
    psAa = env['psAa']; xpool = env['xpool']; stpool = env['stpool']
    x_ext = env['x_ext']; v_bf = env['v_bf']; qk_bf = env['qk_bf']
    v_mid = env['v_mid']; qk_dram = env['qk_dram']
    bv_sb = env['bv_sb']; bqk_sb = env['bqk_sb']

    x_ci = []
    for ci in range(CI):
        t = xpool.tile([P, HW], BF16, tag=f"x{ci}")
        nc.sync.dma_start(out=t[:], in_=x_ext[:, ci, :])
        x_ci.append(t)

    def conv_tile(w_sb, oc, half2):
        # one [128, 4, 512] PSUM tile covering tokens [2048*half2, +2048)
        ps = psAa.tile([P, 4, 512], FP32, tag="s")
        for half in range(4):
            n0 = half2 * 2048 + half * 512
            for ci in range(CI):
                nc.tensor.matmul(
                    ps[:, half, :],
                    w_sb[:, ci, oc * P : (oc + 1) * P],
                    x_ci[ci][:, n0 : n0 + 512],
                    start=(ci == 0),
                    stop=(ci == CI - 1),
                    skip_group_check=True,
                )
        return ps

    # first half of the token range first, so windows 0-1 of the merged
    # phase can start while the second half is still convolving
    for half2 in range(2):
        for oc in range(OC):
            ps = conv_tile(v_bf, oc, half2)
            nc.scalar.activation(
                v_mid[:, oc, half2 * 2048 : (half2 + 1) * 2048],
                ps[:].rearrange("p a b -> p (a b)"),
                SILU,
                bias=bv_sb[:, oc : oc + 1],
            )
        for oc in range(OCQK):
            ps = conv_tile(qk_bf, oc, half2)
            st = stpool.tile([P, 2048], BF16, tag="st")
            nc.scalar.activation(
                st[:], ps[:].rearrange("p a b -> p (a b)"),
                SILU, bias=bqk_sb[:, oc : oc + 1],
            )
            nc.sync.dma_start(
                out=qk_dram[oc * P : (oc + 1) * P,
                            half2 * 2048 : (half2 + 1) * 2048],
                in_=st[:],
            )


def _phase_merged(nc, env):
    pepool = env['pepool']; vaugp = env['vaugp']; a8p = env['a8p']
    qkp = env['qkp']; recipp = env['recipp']; srowp = env['srowp']; tmpp = env['tmpp']
    ppw = env['ppw']; ystp = env['ystp']; attnw = env['attnw']
    pestp = env['pestp']; prstp = env['prstp']; gatep = env['gatep']
    psA = env['psA']; psO = env['psO']; psum_mm = env['psum_mm']
    vlr = env['vlr']; v_mid = env['v_mid']; ident = env['ident']; qk_dram = env['qk_dram']
    dram2 = env['dram2']; pew_ext = env['pew_ext']; pr_bf = env['pr_bf']
    bpe_sb = env['bpe_sb']; bpr_sb = env['bpr_sb']; out_ext = env['out_ext']

    # pr conv results of window g-1, silu'd while window g's exps run
    pending_pr = []

    def flush_pr(gate):
        for pr_st, oc, nch in pending_pr:
            yst = ystp.tile([P, 512], FP32, tag="yst")
            bias = gate[:, oc : oc + 1] if gate is not None else bpr_sb[:, oc : oc + 1]
            nc.scalar.activation(yst[:], pr_st[:], SILU, bias=bias)
            nc.sync.dma_start(
                out=out_ext[oc * P : (oc + 1) * P, nch * 512 : (nch + 1) * 512],
                in_=yst[:],
            )
        pending_pr.clear()

    for g in range(WIN):
        w = g
        # ---- V-aug (fp8) [128 keys, h, jc, 80] with ones column ----
        # row stride 80 so the DoubleRow lhsT kt-dim stride (80 B) is
        # 16-byte aligned (ISA s3_lw_dual_fp8 restriction)
        attn_w = attnw.tile([P, OC, NW], BF16, tag="attnw")
        vaug = vaugp.tile([P, HEADS, JC, 80], FP8, tag="vaug")
        nc.vector.memset(vaug[:, :, :, HD : HD + 1], 1.0)
        for jc in range(JC):
            t0 = w * NW + jc * P
            pt = psum_mm.tile([P, CI, P], BF16, tag="mm")
            for ci in range(CI):
                nc.tensor.matmul(
                    pt[:, ci, :], v_mid[:, ci, t0 : t0 + P], ident[:],
                    is_transpose=True, skip_group_check=True,
                )
            nc.vector.tensor_copy(
                vaug[:, :, jc, 0:HD],
                pt[:].rearrange("p c (s d) -> p (c s) d", s=2),
            )

        # ---- conv3x3 matmuls (oc 0-1 before the heads: they are
        # long-ready PE work that bridges the window boundary while the
        # first exps drain behind the gated silu batch) ----
        og = NW * g - 128                      # tile origin in token space
        ca, cb = max(0, og), min(HW, NW * (g + 1) + 128)
        v_l = vlr.tile([P, CI, NW + 256], BF16, tag="vl")
        v_r = vlr.tile([P, CI, NW + 256], BF16, tag="vr")
        nc.vector.tensor_copy(v_l[:, :, ca - og : cb - og], v_mid[:, :, ca:cb])
        nc.vector.tensor_copy(v_r[:, :, ca - og : cb - og], v_mid[:, :, ca:cb])
        rl = v_l[:].rearrange("p c (r w) -> p c r w", w=64)
        rr = v_r[:].rearrange("p c (r w) -> p c r w", w=64)
        nc.vector.memset(rl[:, :, (ca - og) // 64 : (cb - og) // 64, 63:64], 0)
        nc.vector.memset(rr[:, :, (ca - og) // 64 : (cb - og) // 64, 0:1], 0)
        vtap = {0: v_l, 1: None, 2: v_r}
        pp_w = ppw.tile([P, OC, NW], BF16, tag="ppw")

        def conv3x3_oc(oc):
            pe_sl = pepool.tile([P, CI * 9 * P], BF16, tag="pe")
            nc.sync.dma_start(out=pe_sl[:], in_=pew_ext[oc])
            pe_v = pe_sl[:].rearrange("p (c t o) -> p c t o", c=CI, t=9)
            pe_st = pestp.tile([P, 2, 512], BF16, tag="pest")
            for nch in (2 * g, 2 * g + 1):
                n0 = nch * 512
                ps = psum_mm.tile([P, 512], FP32, tag="mm")
                mms = []
                for ky, kx in TAPS:
                    s = (ky - 1) * 64 + (kx - 1)
                    lo = max(0, -s - n0)
                    hi = min(512, HW - s - n0)
                    for ci in range(CI):
                        if kx == 1:
                            rsl = v_mid[:, ci, n0 + s + lo : n0 + s + hi]
                        else:
                            rsl = vtap[kx][
                                :, ci, n0 + s + lo - og : n0 + s + hi - og
                            ]
                        mms.append((
                            ps[:, lo:hi],
                            pe_v[:, ci, ky * 3 + kx, :],
                            rsl,
                        ))
                for i, (o, l, r) in enumerate(mms):
                    nc.tensor.matmul(
                        o, l, r,
                        start=(i == 0),
                        stop=(i == len(mms) - 1),
                        skip_group_check=True,
                    )
                # evacuate PSUM right away (DVE) so conv matmuls never
                # block on the gated silu
                nc.vector.tensor_copy(pe_st[:, nch - 2 * g, :], ps[:])
            return pe_st

        for h in range(HEADS):
            q_t = qkp.tile([HD, NW], BF16, tag="q")
            nc.sync.dma_start(
                out=q_t[:],
                in_=qk_dram[h * HD : (h + 1) * HD, w * NW : (w + 1) * NW],
            )
            k_t = qkp.tile([HD, NW], BF16, tag="k")
            nc.sync.dma_start(
                out=k_t[:],
                in_=qk_dram[C + h * HD : C + (h + 1) * HD, w * NW : (w + 1) * NW],
            )

            ps_o = psO.tile([HD + 1, NW], FP32, tag="o")
            a8 = None
            for jp in range(JC // 2):
                ps_s = psA.tile([P, 2, NW], FP32, tag="s")
                for jcl in range(2):
                    for half in range(2):
                        nc.tensor.matmul(
                            ps_s[:, jcl, half * 512 : (half + 1) * 512],
                            k_t[:, (2 * jp + jcl) * P : (2 * jp + jcl + 1) * P],
                            q_t[:, half * 512 : (half + 1) * 512],
                            start=True,
                            stop=True,
                            skip_group_check=True,
                        )
                a8 = a8p.tile([P, 2, NW], FP8, tag="a8")
                nc.scalar.activation(
                    a8[:].rearrange("p a b -> p (a b)"),
                    ps_s[:].rearrange("p a b -> p (a b)"),
                    EXP, scale=0.125,
                )
                for half in range(2):
                    nc.tensor.matmul(
                        ps_o[:, half * 512 : (half + 1) * 512],
                        vaug[:, h, 2 * jp : 2 * jp + 2, 0 : HD + 1],
                        a8[:, :, half * 512 : (half + 1) * 512],
                        start=(jp == 0),
                        stop=(jp == JC // 2 - 1),
                        perf_mode=DR,
                        skip_group_check=True,
                    )

            # evict attnV PSUM to SBUF right away so the next head's
            # matmuls aren't blocked behind the normalize chain
            srow = srowp.tile([HD + 1, NW], FP32, tag="srow")
            nc.vector.tensor_copy(srow[:], ps_o[:])
            rrow = dram2.tile([NW], FP32, tag="rrow")
            nc.sync.dma_start(out=rrow[:], in_=srow[HD : HD + 1, :])
            r128 = recipp.tile([P, JC], FP32, tag="r128")
            nc.sync.dma_start(
                out=r128[:], in_=rrow[:].rearrange("(p c) -> p c", p=P)
            )
            nc.vector.reciprocal(r128[:], r128[:])
            rrec = dram2.tile([NW], FP32, tag="rrec")
            nc.sync.dma_start(
                out=rrec[:].rearrange("(p c) -> p c", p=P), in_=r128[:]
            )
            rbc = recipp.tile([HD, NW], FP32, tag="rbc")
            nc.sync.dma_start(
                out=rbc[:], in_=rrec[:].unsqueeze(0).partition_broadcast(HD)
            )
            if h % 2 == 0:
                nc.vector.tensor_mul(
                    attn_w[0:HD, h // 2, :], srow[0:HD, :], rbc[:]
                )
            else:
                tmp = tmpp.tile([HD, NW], BF16, tag="tmp")
                nc.vector.tensor_mul(tmp[:], srow[0:HD, :], rbc[:])
                nc.sync.dma_start(
                    out=attn_w[HD : 2 * HD, h // 2, :], in_=tmp[:]
                )
            if h == HEADS - 1:
                a8_last = a8

        # gate tiles: read one element of the window's final exp output so
        # the conv silus below schedule after all of this window's exps
        # (keeps Exp/Silu act-table loads to ~2 per window). The last
        # window runs ungated so its conv tail overlaps the attention.
        gate_pr = gatep.tile([P, OC], FP32, tag="gpr")
        nc.vector.scalar_tensor_tensor(
            gate_pr[:], a8_last[:, 1, 0:OC], 0.0, bpr_sb[:], MULT, ADD
        )
        flush_pr(gate_pr)
        gate_pe = gatep.tile([P, OC], FP32, tag="gpe")
        nc.vector.scalar_tensor_tensor(
            gate_pe[:], a8_last[:, 1, 0:OC], 0.0, bpe_sb[:], MULT, ADD
        )

        # ---- conv3x3 matmuls, then all gated silus ----
        pe_sts = [conv3x3_oc(oc) for oc in range(OC)]
        for oc, pe_st in enumerate(pe_sts):
            nc.scalar.activation(
                pp_w[:, oc, :],
                pe_st[:].rearrange("p a b -> p (a b)"),
                SILU,
                bias=gate_pe[:, oc : oc + 1],
            )

        # ---- pr conv for this window's tokens ----
        for nloc in range(2):
            nch = 2 * g + nloc
            z = ppw.tile([P, CI, 512], BF16, tag="z")
            nc.vector.tensor_add(
                z[:],
                attn_w[:, :, nloc * 512 : (nloc + 1) * 512],
                pp_w[:, :, nloc * 512 : (nloc + 1) * 512],
            )
            for oc in range(OC):
                ps = psum_mm.tile([P, 512], FP32, tag="mm")
                for ci in range(CI):
                    nc.tensor.matmul(
                        ps[:],
                        pr_bf[:, ci, oc * P : (oc + 1) * P],
                        z[:, ci, :],
                        start=(ci == 0),
                        stop=(ci == CI - 1),
                    )
                pr_st = prstp.tile([P, 512], BF16, tag="prst")
                nc.vector.tensor_copy(pr_st[:], ps[:])
                pending_pr.append((pr_st, oc, nch))

    # last window's pr silus run ungated at the end
    flush_pr(None)


def _build():
    nc = bacc.Bacc(None, target_bir_lowering=False, debug=False)

    x_ext = nc.declare_dram_parameter("x", [P, CI, HW], BF16, isOutput=False)
    qkw_ext = nc.declare_dram_parameter("qk_wt", [P, CI, 2 * C], BF16, isOutput=False)
    vw_ext = nc.declare_dram_parameter("v_wt", [P, CI, C], BF16, isOutput=False)
    pew_ext = nc.declare_dram_parameter("pe_wt", [OC, P, CI * 9 * P], BF16, isOutput=False)
    prw_ext = nc.declare_dram_parameter("pr_wt", [P, CI, C], BF16, isOutput=False)
    bqk_ext = nc.declare_dram_parameter("b_qk", [P, OCQK], FP32, isOutput=False)
    bv_ext = nc.declare_dram_parameter("b_v", [P, OC], FP32, isOutput=False)
    bpe_ext = nc.declare_dram_parameter("b_pe", [P, OC], FP32, isOutput=False)
    bpr_ext = nc.declare_dram_parameter("b_pr", [P, OC], FP32, isOutput=False)
    out_ext = nc.declare_dram_parameter("out", [C, HW], FP32, isOutput=True)

    with TileContext(nc) as tc:
        with (
            tc.tile_pool(name="const", bufs=1) as const_pool,
            tc.tile_pool(name="persist", bufs=1) as persist,
            tc.tile_pool(name="dram", bufs=1, space="DRAM") as dram,
            tc.tile_pool(name="dram2", bufs=3, space="DRAM") as dram2,
        ):
            ident = const_pool.tile([P, P], BF16)
            make_identity(nc, ident)

            bqk_sb = const_pool.tile([P, OCQK], FP32)
            nc.sync.dma_start(out=bqk_sb[:], in_=bqk_ext[:])
            bv_sb = const_pool.tile([P, OC], FP32)
            nc.sync.dma_start(out=bv_sb[:], in_=bv_ext[:])
            bpe_sb = const_pool.tile([P, OC], FP32)
            nc.sync.dma_start(out=bpe_sb[:], in_=bpe_ext[:])
            bpr_sb = const_pool.tile([P, OC], FP32)
            nc.sync.dma_start(out=bpr_sb[:], in_=bpr_ext[:])

            qk_bf = persist.tile([P, CI, 2 * C], BF16)
            nc.sync.dma_start(out=qk_bf[:], in_=qkw_ext[:])
            v_bf = persist.tile([P, CI, C], BF16)
            nc.sync.dma_start(out=v_bf[:], in_=vw_ext[:])
            pr_bf = persist.tile([P, CI, C], BF16)
            nc.sync.dma_start(out=pr_bf[:], in_=prw_ext[:])

            # v feature map (flat) -- attention V source and conv3x3 center
            v_mid = persist.tile([P, OC, HW], BF16)

            qk_dram = dram.tile([2 * C, HW], BF16)

            # phase A gets all 8 PSUM banks for double-buffered
            # [128, 2, 1024] conv tiles
            with (
                tc.tile_pool(name="psAa", bufs=2, space="PSUM") as psAa,
                tc.tile_pool(name="xpool", bufs=1) as xpool,
                tc.tile_pool(name="stpool", bufs=2) as stpool,
            ):
                _phase_a(nc, locals())

            from contextlib import ExitStack

            with ExitStack() as es:
                pools = dict(
                    psum_mm=("PSUM", 2), psA=("PSUM", 1), psO=("PSUM", 1),
                    vlr=(None, 1), attnw=(None, 1), pepool=(None, 2),
                    vaugp=(None, 2), a8p=(None, 4), qkp=(None, 3),
                    srowp=(None, 2), recipp=(None, 2), tmpp=(None, 2),
                    ppw=(None, 2), ystp=(None, 4), pestp=(None, 8),
                    prstp=(None, 10), gatep=(None, 2),
                )
                pvars = {}
                for pname, (space, bufs) in pools.items():
                    kw = {"space": space} if space else {}
                    pvars[pname] = es.enter_context(
                        tc.tile_pool(name=pname, bufs=bufs, **kw)
                    )
                env = dict(locals())
                env.update(pvars)
                _phase_merged(nc, env)

    nc.compile()
    return nc


_NC_CACHE = {}


def _get_nc():
    if "nc" not in _NC_CACHE:
        _NC_CACHE["nc"] = _build()
    return _NC_CACHE["nc"]


def _make_in_maps(inputs):
    import ml_dtypes

    bf16 = ml_dtypes.bfloat16
    x = np.asarray(inputs["x"], dtype=np.float32)          # [8, 512, 64, 64]
    B = x.shape[0]

    def fold(wname, gname, bname, mname, vname):
        g = np.asarray(inputs[gname], np.float32)
        b = np.asarray(inputs[bname], np.float32)
        m = np.asarray(inputs[mname], np.float32)
        v = np.asarray(inputs[vname], np.float32)
        s = g / np.sqrt(v + EPS)
        w = np.asarray(inputs[wname], np.float32)
        return s, (b - m * s).astype(np.float32), w

    s_qk, b_qk, qk_w = fold("qk_w", "qk_g", "qk_b", "qk_rm", "qk_rv")
    s_v, b_v, v_w = fold("v_w", "v_g", "v_b", "v_rm", "v_rv")
    s_pe, b_pe, pe_w = fold("pe_w", "pe_g", "pe_b", "pe_rm", "pe_rv")
    s_pr, b_pr, pr_w = fold("pr_w", "pr_g", "pr_b", "pr_rm", "pr_rv")

    def lhst(w_scaled, o_dim):
        # [O, C] scaled -> [128, CI, O] bf16 (partition = c % 128)
        wt = w_scaled.T.reshape(CI, P, o_dim).transpose(1, 0, 2)
        return np.ascontiguousarray(wt.astype(bf16))

    qk_wt = lhst(qk_w * s_qk[:, None], 2 * C)
    v_wt = lhst(v_w * s_v[:, None], C)
    pr_wt = lhst(pr_w * s_pr[:, None], C)

    # pe: [O, C, 3, 3] -> per oc chunk: [128(c%128), CI, 9, 128(o)] bf16
    pe = (pe_w * s_pe[:, None, None, None]).transpose(2, 3, 1, 0)  # ky,kx,c,o
    pe = pe.reshape(9, CI, P, OC, P)            # tap, ci, p, oc, op
    pe = pe.transpose(3, 2, 1, 0, 4)            # oc, p, ci, tap, op
    pe_wt = np.ascontiguousarray(pe.reshape(OC, P, CI * 9 * P).astype(bf16))

    def bias_r(b, n):
        return np.ascontiguousarray(b.reshape(n, P).T)

    shared = {
        "qk_wt": qk_wt, "v_wt": v_wt, "pe_wt": pe_wt, "pr_wt": pr_wt,
        "b_qk": bias_r(b_qk, OCQK), "b_v": bias_r(b_v, OC),
        "b_pe": bias_r(b_pe, OC), "b_pr": bias_r(b_pr, OC),
    }
    xs = x.reshape(B, CI, P, HW).transpose(0, 2, 1, 3).astype(bf16)
    return [
        {"x": np.ascontiguousarray(xs[i]), **shared}
        for i in range(B)
    ]


def kernel(**inputs):
    from concourse.bass_utils import run_bass_kernel_spmd

    in_maps = _make_in_maps(inputs)
    B = len(in_maps)
    nc = _get_nc()
    res = run_bass_kernel_spmd(nc, in_maps, core_ids=list(range(B)))
    out = np.stack([res.results[i]["out"] for i in range(B)], axis=0)
    return out.reshape(B, C, 64, 64).astype(np.float32)
